# revision 1
# baseline (speedup 1.0000x reference)
"""CVKAN layer Trainium2 kernel.

Math (per reference):
    basis[b, i, k] = exp(-((x_part[b,i] - grid[k%8]) / h)^2), part = re if k<8 else im
    out_re[b, o]   = sum_{i,k} basis[b,i,k] * coeffs_re[i,o,k] + bias_re[o]
    out_im[b, o]   = sum_{i,k} basis[b,i,k] * coeffs_im[i,o,k] + bias_im[o]
    out = out_re + 1j*out_im   (complex64)

Device strategy (pure data-parallel over batch across 8 cores, no
collectives needed):
  - Load x tiles [128b, 128(i_re|i_im)] and PE-transpose each 128x128 block
    so the contraction index (part, i) sits on partitions: T [128, b].
  - For each grid point j (8 per part): one contraction chunk.
    basis_j = (2/sqrt(pi))*exp(-z^2), z = (T - g_j)/h, evaluated in a
    single ScalarE pass via Derivative_Erf (the 2/sqrt(pi) prefactor is
    folded into the weights host-side). The grid shift is the activation's
    free affine bias, so each chunk differs only in a per-partition bias
    column.
  - TensorE accumulates out^T[32, b] += W_j^T @ basis_j with the small
    weight matrix stationary and basis streaming as float32r (single-pass
    fp32 streaming, 4x faster than exact fp32 matmul; ~2e-4 output rel err).
  - The complex bias is added during the PSUM->SBUF eviction as a
    per-partition tensor_scalar add on VectorE.
  - out^T [32, 8192] fp32 per core is stored contiguously; the host
    interleaves re/im into complex64 while gathering the batch shards.
  - Tile sizes are graduated (small first tile so ScalarE starts early,
    small last tile so the matmul/store tail after the final activation is
    short). ScalarE is the bottleneck engine (~63us busy of ~77us total);
    TensorE/VectorE/DMA run underneath it.
"""

import sys

import numpy as np

if "/opt/trn_rl_repo" not in sys.path:
    sys.path.append("/opt/trn_rl_repo")

B = 65536
IN = 64
OUT = 16
NB = 8
N_CORES = 8
B_CORE = B // N_CORES  # 8192
H = 2.0 / (NB - 1)
GRID = [-1.0 + j * H for j in range(NB)]

# Graduated tile sizes: small first tile starts ScalarE sooner; small last
# tile shortens the matmul/copy/store tail after the final activation.
TILE_SIZES = [1024, 2048, 2048, 2048, 1024]
assert sum(TILE_SIZES) == B_CORE

_CACHE = {}


def _build_module():
    import concourse.mybir as mybir
    import concourse.tile as tile
    from concourse import bacc
    from concourse.masks import make_identity

    f32 = mybir.dt.float32
    f32r = mybir.dt.float32r
    nc = bacc.Bacc("TRN2", target_bir_lowering=False, debug=False,
                   num_devices=N_CORES)

    x_re = nc.dram_tensor("x_re", [B_CORE, IN], f32, kind="ExternalInput")
    x_im = nc.dram_tensor("x_im", [B_CORE, IN], f32, kind="ExternalInput")
    w = nc.dram_tensor("w", [NB, 128, 2 * OUT], f32r, kind="ExternalInput")
    bias32 = nc.dram_tensor("bias32", [1, 2 * OUT], f32, kind="ExternalInput")
    out_t = nc.dram_tensor("out_t", [2 * OUT, B_CORE], f32,
                           kind="ExternalOutput")

    DErf = mybir.ActivationFunctionType.Derivative_Erf

    with tile.TileContext(nc) as tc:
        with (
            tc.tile_pool(name="consts", bufs=1) as consts,
            tc.tile_pool(name="xin", bufs=6) as xpool,
            tc.tile_pool(name="tpsum", bufs=3, space="PSUM") as tpsum,
            tc.tile_pool(name="tsb", bufs=4) as tpool,
            tc.tile_pool(name="basis", bufs=8) as bpool,
            tc.tile_pool(name="opsum", bufs=1, space="PSUM") as opsum,
            tc.tile_pool(name="osb", bufs=3) as opool,
        ):
            identity = consts.tile([128, 128], f32)
            make_identity(nc, identity)
            w_sb = consts.tile([128, NB * 2 * OUT], f32r)
            nc.sync.dma_start(
                out=w_sb[:].rearrange("p (j o) -> p j o", j=NB),
                in_=w.ap().rearrange("j p o -> p j o"),
            )
            bias_sb = consts.tile([2 * OUT, 1], f32)
            nc.sync.dma_start(out=bias_sb[:],
                              in_=bias32.ap().rearrange("a o -> o a"))
            # Per-chunk activation bias columns: bias_j = -grid[j]/h.
            gbias = consts.tile([128, NB], f32)
            for j in range(NB):
                nc.vector.memset(gbias[:, j:j + 1], -GRID[j] / H)

            def build_T(g):
                bt = TILE_SIZES[g]
                base = sum(TILE_SIZES[:g])
                T = tpool.tile([128, bt], f32, tag="T")
                for q in range(bt // 512):
                    # Load 4 b-blocks (512 batch rows) of x_re|x_im columns.
                    xcat = xpool.tile([128, 512], f32)
                    xv = xcat[:].rearrange("p (nb c) -> p nb c", c=128)
                    b0 = base + q * 512
                    nc.sync.dma_start(
                        out=xv[:, :, 0:IN],
                        in_=x_re.ap()[b0:b0 + 512, :]
                            .rearrange("(nb p) i -> p nb i", p=128),
                    )
                    nc.sync.dma_start(
                        out=xv[:, :, IN:128],
                        in_=x_im.ap()[b0:b0 + 512, :]
                            .rearrange("(nb p) i -> p nb i", p=128),
                    )
                    tp = tpsum.tile([128, 512], f32)
                    for r in range(4):
                        nc.tensor.transpose(
                            tp[:, r * 128:(r + 1) * 128],
                            xcat[:, r * 128:(r + 1) * 128],
                            identity,
                        )
                    nc.vector.tensor_copy(T[:, q * 512:(q + 1) * 512], tp[:])
                return T

            base = 0
            nextT = build_T(0)
            for g, bt in enumerate(TILE_SIZES):
                T = nextT
                out_ps = opsum.tile([2 * OUT, bt], f32, tag="out_ps")
                # Hoist the next tile's load/transpose/copy ahead of this
                # tile's activations so its T is ready the moment ScalarE
                # finishes the current tile (the copies would otherwise queue
                # behind this tile's store ops on VectorE).
                if g + 1 < len(TILE_SIZES):
                    nextT = build_T(g + 1)
                for j in range(NB):
                    basis = bpool.tile([128, bt], f32r, tag="basis")
                    nc.scalar.activation(basis[:], T[:], DErf,
                                         bias=gbias[:, j:j + 1],
                                         scale=1.0 / H)
                    for s in range(bt // 512):
                        nc.tensor.matmul(
                            out_ps[:, s * 512:(s + 1) * 512],
                            w_sb[:, j * 2 * OUT:(j + 1) * 2 * OUT],
                            basis[:, s * 512:(s + 1) * 512],
                            start=(j == 0),
                            stop=(j == NB - 1),
                        )
                out_sb = opool.tile([2 * OUT, bt], f32, tag="out_sb")
                ostep = min(1024, bt)
                for u in range(bt // ostep):
                    sl = slice(u * ostep, (u + 1) * ostep)
                    nc.vector.tensor_scalar_add(out_sb[:, sl], out_ps[:, sl],
                                                bias_sb[:])
                    nc.sync.dma_start(
                        out=out_t.ap()[:, base + u * ostep:
                                       base + (u + 1) * ostep],
                        in_=out_sb[:, sl],
                    )
                base += bt

    nc.compile()
    return nc


def _get_module():
    if "nc" not in _CACHE:
        _CACHE["nc"] = _build_module()
    return _CACHE["nc"]


def _build_w(coeffs_re, coeffs_im):
    w = np.empty((NB, 128, 2 * OUT), dtype=np.float32)
    w[:, :IN, :OUT] = np.transpose(coeffs_re[:, :, :NB], (2, 0, 1))
    w[:, :IN, OUT:] = np.transpose(coeffs_im[:, :, :NB], (2, 0, 1))
    w[:, IN:, :OUT] = np.transpose(coeffs_re[:, :, NB:], (2, 0, 1))
    w[:, IN:, OUT:] = np.transpose(coeffs_im[:, :, NB:], (2, 0, 1))
    # Fold the Derivative_Erf prefactor 2/sqrt(pi) into the weights.
    w *= np.float32(np.sqrt(np.pi) / 2.0)
    return w


def kernel(x_re, x_im, coeffs_re, coeffs_im, bias_re, bias_im):
    from concourse.bass_utils import run_bass_kernel_spmd

    nc = _get_module()
    w = _build_w(np.asarray(coeffs_re), np.asarray(coeffs_im))
    bias32 = np.concatenate(
        [np.asarray(bias_re), np.asarray(bias_im)]
    ).astype(np.float32).reshape(1, 2 * OUT)

    x_re = np.ascontiguousarray(x_re, dtype=np.float32)
    x_im = np.ascontiguousarray(x_im, dtype=np.float32)
    in_maps = [
        {
            "x_re": x_re[c * B_CORE:(c + 1) * B_CORE],
            "x_im": x_im[c * B_CORE:(c + 1) * B_CORE],
            "w": w,
            "bias32": bias32,
        }
        for c in range(N_CORES)
    ]
    res = run_bass_kernel_spmd(nc, in_maps, core_ids=list(range(N_CORES)))
    out = np.empty((B, OUT), dtype=np.complex64)
    for c in range(N_CORES):
        ot = res.results[c]["out_t"]  # [32, B_CORE] fp32
        out[c * B_CORE:(c + 1) * B_CORE] = (ot[:OUT].T + 1j * ot[OUT:].T)
    return out



# revision 2
# speedup vs baseline: 1.5012x; 1.5012x over previous
"""CVKAN layer Trainium2 kernel (v3: recurrence basis + stationary-basis matmul).

Math (per reference):
    u = (x + 1)/h,  h = 2/(NB-1) = 2/7
    basis_j(x) = exp(-(u - j)^2)                       j = 0..7, per part (re|im)
    out[b, o]  = sum_{p=(part,i), j} basis_j(x_p[b]) * w[p, j, o] + bias[o]

Gaussian recurrence: basis_{j0+m} = seed_{j0} * r^m * exp(-(2*j0*m + m^2)),
with seed_{j0} = (2/sqrt(pi)) exp(-(u-j0)^2) via ScalarE Derivative_Erf and
r = exp(2u) via ScalarE Exp.  The constant factors are folded into the
weights host-side, so each derived basis is ONE bf16 multiply.

Device strategy (8 cores, pure batch data-parallel, no collectives):
  - Host concatenates re|im into x2 [B_CORE, 128] bf16; XBAR dma_start_transpose
    loads T [128, chunk] directly (no PE transposes, no PSUM staging).
  - ScalarE: 3 passes per chunk (seed0, seed4, r), bf16 out.
  - 6 derived basis multiplies, column-striped: DVE takes 13/16 of columns
    (bf16 2x_1p mode, ~0.52 ns/col), Pool the rest (~1.98 ns/col).
  - PE: per 128-batch block, 8 accumulating matmuls with the basis block as
    the STATIONARY operand and the small W_j [128, 32] as the moving tensor:
    cost is 32 output rows per matmul instead of 128+ streamed columns.
    PSUM collects out[b, o] directly - output needs no final transpose.
  - DVE evicts PSUM [128, nblk*32] -> SBUF once per chunk; DMA stores
    y [B_CORE, 32] fp32; host assembles complex64 and adds the bias.
"""

import sys

import numpy as np

if "/opt/trn_rl_repo" not in sys.path:
    sys.path.append("/opt/trn_rl_repo")

B = 65536
IN = 64
OUT = 16
NB = 8
N_CORES = 8
B_CORE = B // N_CORES  # 8192
H = 2.0 / (NB - 1)

# Graduated chunk sizes (batch rows per pipeline stage). Small first chunk
# so ScalarE starts early; small last chunk to shorten the store tail.
CHUNKS = [1024, 2048, 2048, 2048, 1024]
assert sum(CHUNKS) == B_CORE
# Fraction of columns the DVE takes for the derived-basis multiplies
# (rest go to Pool/GpSimd). Rounded to whole 128-col matmul blocks.
DVE_FRAC = 13.0 / 16.0

_CACHE = {}


def _build_module():
    import concourse.mybir as mybir
    import concourse.tile as tile
    from concourse import bacc

    f32 = mybir.dt.float32
    bf16 = mybir.dt.bfloat16
    nc = bacc.Bacc("TRN2", target_bir_lowering=False, debug=False,
                   num_devices=N_CORES)

    x2 = nc.dram_tensor("x2", [B_CORE, 128], bf16, kind="ExternalInput")
    w = nc.dram_tensor("w", [128, NB * 2 * OUT], bf16, kind="ExternalInput")
    y = nc.dram_tensor("y", [B_CORE, 2 * OUT], f32, kind="ExternalOutput")

    DErf = mybir.ActivationFunctionType.Derivative_Erf
    Exp = mybir.ActivationFunctionType.Exp
    Mult = mybir.AluOpType.mult

    with tile.TileContext(nc) as tc:
        with (
            tc.tile_pool(name="consts", bufs=1) as consts,
            tc.tile_pool(name="tin", bufs=2) as tpool,
            tc.tile_pool(name="basis", bufs=2) as bpool,
            tc.tile_pool(name="opsum", bufs=2, space="PSUM") as opsum,
            tc.tile_pool(name="osb", bufs=2) as opool,
        ):
            w_sb = consts.tile([128, NB * 2 * OUT], bf16)
            nc.sync.dma_start(out=w_sb[:], in_=w.ap())
            # Activation bias columns: seed0 -> -g0/H, seed4 -> -g4/H, r -> 2/H.
            gbias = consts.tile([128, 3], f32)
            nc.vector.memset(gbias[:, 0:1], 1.0 / H)          # -(-1)/H
            nc.vector.memset(gbias[:, 1:2], -((-1.0 + 4 * H) / H))
            nc.vector.memset(gbias[:, 2:3], 2.0 / H)

            base = 0
            for g, cn in enumerate(CHUNKS):
                nblk = cn // 128
                T = tpool.tile([128, cn], bf16, tag="T")
                nc.sync.dma_start_transpose(
                    out=T[:], in_=x2.ap()[base:base + cn, :])

                bj = [None] * NB
                r = bpool.tile([128, cn], bf16, tag="r")
                nc.scalar.activation(r[:], T[:], Exp,
                                     bias=gbias[:, 2:3], scale=2.0 / H)
                for j0 in (0, 4):
                    seed = bpool.tile([128, cn], bf16, tag=f"b{j0}")
                    nc.scalar.activation(seed[:], T[:], DErf,
                                         bias=gbias[:, j0 // 4:j0 // 4 + 1],
                                         scale=1.0 / H)
                    bj[j0] = seed
                # Derived basis: two independent chains (0->1->2->3, 4->5->6->7),
                # each link column-striped across DVE and Pool.
                cd = 128 * int(round(DVE_FRAC * nblk))  # DVE stripe width
                for m in (1, 2, 3):
                    for j0 in (0, 4):
                        j = j0 + m
                        bt = bpool.tile([128, cn], bf16, tag=f"b{j}")
                        nc.vector.tensor_tensor(
                            bt[:, 0:cd], bj[j - 1][:, 0:cd], r[:, 0:cd], Mult)
                        if cd < cn:
                            nc.gpsimd.tensor_tensor(
                                bt[:, cd:cn], bj[j - 1][:, cd:cn],
                                r[:, cd:cn], Mult)
                        bj[j] = bt

                out_ps = opsum.tile([128, nblk * 2 * OUT], f32, tag="out_ps")
                for k in range(nblk):
                    ks = slice(k * 2 * OUT, (k + 1) * 2 * OUT)
                    for j in range(NB):
                        nc.tensor.matmul(
                            out_ps[:, ks],
                            bj[j][:, k * 128:(k + 1) * 128],
                            w_sb[:, j * 2 * OUT:(j + 1) * 2 * OUT],
                            start=(j == 0),
                            stop=(j == NB - 1),
                        )
                out_sb = opool.tile([128, nblk * 2 * OUT], f32, tag="out_sb")
                nc.vector.tensor_copy(out_sb[:], out_ps[:])
                nc.sync.dma_start(
                    out=y.ap()[base:base + cn, :]
                        .rearrange("(g p) o -> p g o", p=128),
                    in_=out_sb[:].rearrange("p (g o) -> p g o", g=nblk),
                )
                base += cn

    nc.compile()
    return nc


def _get_module():
    if "nc" not in _CACHE:
        _CACHE["nc"] = _build_module()
    return _CACHE["nc"]


def _build_w(coeffs_re, coeffs_im):
    import ml_dtypes

    # w2[p, j, o]: p = (part, i) contraction index, j = grid index within
    # part, o = (re outputs | im outputs). Scale folds the Derivative_Erf
    # prefactor sqrt(pi)/2 and the recurrence constant exp(-(2*j0*m + m^2)).
    w2 = np.empty((128, NB, 2 * OUT), dtype=np.float64)
    for j in range(NB):
        w2[:IN, j, :OUT] = coeffs_re[:, :, j]
        w2[:IN, j, OUT:] = coeffs_im[:, :, j]
        w2[IN:, j, :OUT] = coeffs_re[:, :, NB + j]
        w2[IN:, j, OUT:] = coeffs_im[:, :, NB + j]
        j0 = 0 if j < 4 else 4
        m = j - j0
        w2[:, j, :] *= np.sqrt(np.pi) / 2.0 * np.exp(-(2.0 * j0 * m + m * m))
    return w2.reshape(128, NB * 2 * OUT).astype(ml_dtypes.bfloat16)


def kernel(x_re, x_im, coeffs_re, coeffs_im, bias_re, bias_im):
    import ml_dtypes
    from concourse.bass_utils import run_bass_kernel_spmd

    nc = _get_module()
    w = _build_w(np.asarray(coeffs_re, dtype=np.float64),
                 np.asarray(coeffs_im, dtype=np.float64))
    x2 = np.concatenate(
        [np.asarray(x_re, dtype=np.float32),
         np.asarray(x_im, dtype=np.float32)], axis=1
    ).astype(ml_dtypes.bfloat16)

    in_maps = [
        {"x2": np.ascontiguousarray(x2[c * B_CORE:(c + 1) * B_CORE]), "w": w}
        for c in range(N_CORES)
    ]
    res = run_bass_kernel_spmd(nc, in_maps, core_ids=list(range(N_CORES)))
    out = np.empty((B, OUT), dtype=np.complex64)
    for c in range(N_CORES):
        yc = res.results[c]["y"]  # [B_CORE, 32] fp32
        out[c * B_CORE:(c + 1) * B_CORE] = yc[:, :OUT] + 1j * yc[:, OUT:]
    out += (np.asarray(bias_re) + 1j * np.asarray(bias_im)).astype(np.complex64)
    return out


# revision 8
# speedup vs baseline: 1.5966x; 1.0635x over previous
"""CVKAN layer Trainium2 kernel (v4: single-act-table Gaussian cascade).

Math (per reference):
    u = (x + 1)/h,  h = 2/(NB-1) = 2/7
    basis_j(x) = exp(-(u - j)^2)                  j = 0..7, per part (re|im)
    out[b, o]  = sum_{p=(part,i), j} basis_j(x_p[b]) * w[p, j, o] + bias[o]

Single-seed cascade, all activations from ONE table (exp_and_others:
Square, Exp, Copy) so there are zero activation-table reloads:
    sq    = Square(x/h + 1/h)          = u^2            (fp32)
    b'_0  = Exp(-sq + C)               = e^{C - u^2}    (bf16, C=30 boost)
    r     = Exp(2x/h + 2/h)            = e^{2u}         (bf16)
    b'_j  = b'_{j-1} * r                                (bf16 multiplies)
    basis_j = b'_j * e^{-j^2 - C}      (constant folded into the weights)

Device strategy (8 cores, pure batch data-parallel, no collectives):
  - Host concatenates re|im into x2 [B_CORE, 128] bf16; XBAR dma_start_transpose
    loads T [128, chunk] directly (no PE transposes).
  - ScalarE: 3 passes per chunk (sq, seed, r) + the PSUM->SBUF eviction copy.
  - 7 cascade multiplies per chunk, column-striped: DVE takes 13/16 of the
    columns (bf16 2x_1p mode ~0.52 ns/col), Pool/GpSimd the rest.
  - PE: j-major accumulating matmuls with the basis block [128,128] as the
    STATIONARY operand and W_j [128, 32] moving: 32 output rows per matmul,
    and after the last cascade link only the j=7 matmuls remain (short tail).
    PSUM collects out[b, o] directly - no output transpose.
  - Evictions are software-pipelined one chunk behind the activations so the
    ScalarE queue never stalls on matmul completion.
"""

import sys

import numpy as np

if "/opt/trn_rl_repo" not in sys.path:
    sys.path.append("/opt/trn_rl_repo")

B = 65536
IN = 64
OUT = 16
NB = 8
N_CORES = 8
B_CORE = B // N_CORES  # 8192
H = 2.0 / (NB - 1)
CBOOST = 30.0

# Graduated chunk sizes (batch rows per pipeline stage). nblk = cn/128 must
# stay <= 16 so each chunk's PSUM tile [128, nblk*32] f32 fits one 2KB bank.
CHUNKS = [1024, 2048, 2048, 2048, 1024]
assert sum(CHUNKS) == B_CORE
# Fraction of link columns the DVE takes (rest on Pool), in 128-col blocks.
DVE_FRAC = 13.0 / 16.0

_CACHE = {}


def _build_module():
    import concourse.mybir as mybir
    import concourse.tile as tile
    from concourse import bacc

    f32 = mybir.dt.float32
    bf16 = mybir.dt.bfloat16
    nc = bacc.Bacc("TRN2", target_bir_lowering=False, debug=False,
                   num_devices=N_CORES)

    x2 = nc.dram_tensor("x2", [B_CORE, 128], bf16, kind="ExternalInput")
    w = nc.dram_tensor("w", [128, NB * 2 * OUT], bf16, kind="ExternalInput")
    y = nc.dram_tensor("y", [B_CORE, 2 * OUT], f32, kind="ExternalOutput")

    Square = mybir.ActivationFunctionType.Square
    Exp = mybir.ActivationFunctionType.Exp
    Copy = mybir.ActivationFunctionType.Copy
    Mult = mybir.AluOpType.mult

    with tile.TileContext(nc) as tc:
        with (
            tc.tile_pool(name="consts", bufs=1) as consts,
            tc.tile_pool(name="tin", bufs=2) as tpool,
            tc.tile_pool(name="sq", bufs=2) as spool,
            tc.tile_pool(name="basis", bufs=2) as bpool,
            tc.tile_pool(name="opsum", bufs=1, space="PSUM") as opsum,
            tc.tile_pool(name="osb", bufs=2) as opool,
        ):
            w_sb = consts.tile([128, NB * 2 * OUT], bf16)
            nc.sync.dma_start(out=w_sb[:], in_=w.ap())
            # Per-partition bias columns (floats need pre-registered consts).
            gbias = consts.tile([128, 4], f32)
            nc.vector.memset(gbias[:, 0:1], 1.0 / H)
            nc.vector.memset(gbias[:, 1:2], CBOOST)
            nc.vector.memset(gbias[:, 2:3], 2.0 / H)
            nc.vector.memset(gbias[:, 3:4], 0.0)

            pending = []  # (psum, nblk, base) awaiting evict + store

            def flush_one():
                ps, nblk, base0 = pending.pop(0)
                out_sb = opool.tile([128, nblk * 2 * OUT], f32, tag="out_sb")
                nc.scalar.activation(out_sb[:], ps[:], Copy)
                nc.sync.dma_start(
                    out=y.ap()[base0:base0 + nblk * 128, :]
                        .rearrange("(g p) o -> p g o", p=128),
                    in_=out_sb[:].rearrange("p (g o) -> p g o", g=nblk),
                )

            base = 0
            for g, cn in enumerate(CHUNKS):
                nblk = cn // 128
                T = tpool.tile([128, cn], bf16, tag="T")
                nc.sync.dma_start_transpose(
                    out=T[:], in_=x2.ap()[base:base + cn, :])

                sq = spool.tile([128, cn], f32, tag="sq")
                nc.scalar.activation(sq[:], T[:], Square,
                                     bias=gbias[:, 0:1], scale=1.0 / H)
                bj = [None] * NB
                seed = bpool.tile([128, cn], bf16, tag="b0")
                nc.scalar.activation(seed[:], sq[:], Exp,
                                     bias=gbias[:, 1:2], scale=-1.0)
                bj[0] = seed
                r = bpool.tile([128, cn], bf16, tag="r")
                nc.scalar.activation(r[:], T[:], Exp,
                                     bias=gbias[:, 2:3], scale=2.0 / H)

                # Software-pipelined evict of the previous chunk: emitted
                # after this chunk's activations so ScalarE keeps streaming.
                if pending:
                    flush_one()

                cd = 128 * int(round(DVE_FRAC * nblk))  # DVE stripe width
                for m in range(1, NB):
                    bt = bpool.tile([128, cn], bf16, tag=f"b{m}")
                    nc.vector.tensor_tensor(
                        bt[:, 0:cd], bj[m - 1][:, 0:cd], r[:, 0:cd], Mult)
                    if cd < cn:
                        nc.gpsimd.tensor_tensor(
                            bt[:, cd:cn], bj[m - 1][:, cd:cn],
                            r[:, cd:cn], Mult)
                    bj[m] = bt

                out_ps = opsum.tile([128, nblk * 2 * OUT], f32,
                                    tag=f"ps{g}", bufs=1)
                for k in range(nblk):
                    for j in range(NB):
                        nc.tensor.matmul(
                            out_ps[:, k * 2 * OUT:(k + 1) * 2 * OUT],
                            bj[j][:, k * 128:(k + 1) * 128],
                            w_sb[:, j * 2 * OUT:(j + 1) * 2 * OUT],
                            start=(j == 0),
                            stop=(j == NB - 1),
                        )
                pending.append((out_ps, nblk, base))
                base += cn

            while pending:
                flush_one()

    nc.compile()
    return nc


def _get_module():
    if "nc" not in _CACHE:
        _CACHE["nc"] = _build_module()
    return _CACHE["nc"]


def _build_w(coeffs_re, coeffs_im):
    import ml_dtypes

    # w2[p, j, o]: p = (part, i) contraction index, j = grid index within
    # part, o = (re outputs | im outputs). Scale folds the cascade constant
    # exp(-j^2 - C).
    w2 = np.empty((128, NB, 2 * OUT), dtype=np.float64)
    for j in range(NB):
        w2[:IN, j, :OUT] = coeffs_re[:, :, j]
        w2[:IN, j, OUT:] = coeffs_im[:, :, j]
        w2[IN:, j, :OUT] = coeffs_re[:, :, NB + j]
        w2[IN:, j, OUT:] = coeffs_im[:, :, NB + j]
        w2[:, j, :] *= np.exp(-float(j * j) - CBOOST)
    return w2.reshape(128, NB * 2 * OUT).astype(ml_dtypes.bfloat16)


def kernel(x_re, x_im, coeffs_re, coeffs_im, bias_re, bias_im):
    import ml_dtypes
    from concourse.bass_utils import run_bass_kernel_spmd

    nc = _get_module()
    w = _build_w(np.asarray(coeffs_re, dtype=np.float64),
                 np.asarray(coeffs_im, dtype=np.float64))
    x2 = np.concatenate(
        [np.asarray(x_re, dtype=np.float32),
         np.asarray(x_im, dtype=np.float32)], axis=1
    ).astype(ml_dtypes.bfloat16)

    in_maps = [
        {"x2": np.ascontiguousarray(x2[c * B_CORE:(c + 1) * B_CORE]), "w": w}
        for c in range(N_CORES)
    ]
    res = run_bass_kernel_spmd(nc, in_maps, core_ids=list(range(N_CORES)))
    out = np.empty((B, OUT), dtype=np.complex64)
    for c in range(N_CORES):
        yc = res.results[c]["y"]  # [B_CORE, 32] fp32
        out[c * B_CORE:(c + 1) * B_CORE] = yc[:, :OUT] + 1j * yc[:, OUT:]
    out += (np.asarray(bias_re) + 1j * np.asarray(bias_im)).astype(np.complex64)
    return out
